# revision 20
# baseline (speedup 1.0000x reference)
"""MoE top-2 routed FFN (B=4, S=2048, D=1024, H=2048, E=8) on 8 TRN2 NeuronCores.

Strategy (expert-parallel, matching the sharding hint):
  - Host computes the tiny gate (softmax top-2) and builds per-expert token
    lists ("all-to-all dispatch" done at the sharding step).
  - Core e receives the tokens routed to expert e (gathered, transposed,
    zero-padded to capacity C), plus expert e's weights pre-packed into the
    exact tile layouts the kernel consumes.
  - Each core runs a dense FFN  out = coef * ((relu(x@W1.T)^2 * (x@W3.T)) @ W2.T)
    over its C tokens.  Matmuls run in bf16 with fp32 PSUM accumulation,
    except h-tiles 14,15 of the phase-2 contraction, which go through one
    fp8e4 DoubleRow matmul per output tile (2 contraction tiles per pass,
    ~2x rate; h scaled by 1/4 and W2 by 4 so the pair's partial product is
    exact in the shared accumulation).  Error scales with sqrt(fp8 fraction):
    one pair of 16 = measured end-to-end rel err 1.40e-2 (gate 2e-2);
    coefficients stay fp32, outputs bf16.
  - Host scatter-adds the per-expert outputs back ("combine").

Per-core kernel structure (single pass, weights read once):
  phase 1: for each of 16 H-tiles m: psA = W1m @ xT, psB = W3m @ xT (PSUM),
           gT[m] = relu(psA)^2 * psB  (DVE, bf16)   [H, C] layout
  phase 2: for each 128-token tile: out[tok, :] = (gT.T @ W2T) * coef  (PSUM->DVE->DRAM)

Perf notes:
  - Dep-free warmup (36 N=128 matmuls off a memset tile) starts at ~7us and
    flips the HAM clock gate (1.2->2.4GHz) while the first input DMAs are in
    flight; sized to drain right as the first real matmul's data lands.
  - Few, large input DMAs on the SP HWDGE ring in consumption order (W1|W3
    packed per m-tile, x tail groups as single 3D DMAs, W2 as one 4MB DMA).
    Concurrent DMA queues share SDMA bandwidth equally, so later-needed
    groups are interleaved to not starve the group-0 head path; w13 pool
    reuse (bufs=3) compute-paces the later weight DMAs on the serial ring.
  - Output DMAs ride the otherwise-idle ACT HWDGE ring, in bf16, and the
    final token tile's quarter-pieces land in a contiguous staging tensor
    (strided [:,h*128:(h+1)*128] DRAM writes were descriptor-bound, ~9us of
    tail); ob_pool bufs=6 avoids a WAR stall on the last quarter's scale.
"""

import os
import sys

import numpy as np

if os.path.isdir("/opt/trn_rl_repo") and "/opt/trn_rl_repo" not in sys.path:
    sys.path.insert(0, "/opt/trn_rl_repo")

import ml_dtypes

import concourse.bacc as bacc
import concourse.mybir as mybir
from concourse.bass_utils import run_bass_kernel_spmd
from concourse.tile import TileContext

B, S, D, H, E = 4, 2048, 1024, 2048, 8
N = B * S
P = 128
KT = D // P   # 8 contraction tiles over D
MT = H // P   # 16 tiles over H

F32 = mybir.dt.float32
BF16 = mybir.dt.bfloat16
F8 = mybir.dt.float8e4
BF16_NP = ml_dtypes.bfloat16
F8_NP = ml_dtypes.float8_e4m3fn
MT_BF = 14   # h-tiles 0..13 stay bf16; tiles 14,15 go fp8 via one DoubleRow MM

# Set by test harness to capture profiling info.
TRACE = False
LAST_RESULTS = None


def _token_groups(c0, cw):
    """Split [c0, c0+cw) into moving-dim groups of at most 512."""
    groups = []
    rem = cw
    off = c0
    while rem > 0:
        if 512 < rem < 768:
            g = max(min(rem - 256, 512), 256)
        else:
            g = min(512, rem)
        groups.append((off, g))
        off += g
        rem -= g
    return groups


def build_kernel(C):
    TT = (C + P - 1) // P
    TL = C - (TT - 1) * P  # last token tile width (<=128)
    CP = ((C + 15) // 16) * 16  # fp8 pair-tile stride (step%16==0)
    groups = _token_groups(0, C)
    HEAD = groups[0][1]  # first group, served by per-k head tiles

    nc = bacc.Bacc("TRN2", target_bir_lowering=False)

    # x transposed, token-major-last: xt2[p, k, c] = x_tok[c, k*128+p]
    xt2 = nc.dram_tensor("xt2", [P, KT, C], BF16, kind="ExternalInput")
    # w13p[m][d, 0, k*128+h] = W1[m*128+h, k*128+d]; [:,1,:] same for W3
    w13p = nc.dram_tensor("w13p", [MT, P, 2, KT * P], BF16, kind="ExternalInput")
    # 32KB duplicate of w13p[0,:,0,:128] so the first matmul isn't gated on
    # the full 512KB m=0 weight transfer
    w1k0 = nc.dram_tensor("w1k0", [P, P], BF16, kind="ExternalInput")
    # w2a[p, hk, d] = W2[d, hk*128+p]  (bf16 h-tiles 0..13 only)
    w2a = nc.dram_tensor("w2a", [P, MT_BF * D], BF16, kind="ExternalInput")
    # fp8 pair for h-tiles 14,15: w2dr[p, j, d] = W2[d, (14+j)*128+p] * 4
    w2dr = nc.dram_tensor("w2dr", [P, 2, D], F8, kind="ExternalInput")
    cf = nc.dram_tensor("cf", [P, TT], F32, kind="ExternalInput")
    out = nc.dram_tensor("out", [TT, 2, P, 512], BF16, kind="ExternalOutput")
    # contiguous staging for the final (t, dg=1) quarter pieces
    out2 = nc.dram_tensor("out2", [4, P, 128], BF16, kind="ExternalOutput")

    with TileContext(nc) as tc:
        with (
            tc.tile_pool(name="xh_pool", bufs=KT) as xh_pool,
            tc.tile_pool(name="xg_pool", bufs=1) as xg_pool,
            tc.tile_pool(name="g_pool", bufs=1) as g_pool,
            tc.tile_pool(name="w13_pool", bufs=3) as w13_pool,
            tc.tile_pool(name="w2_pool", bufs=1) as w2_pool,
            tc.tile_pool(name="tmp_pool", bufs=2) as tmp_pool,
            tc.tile_pool(name="ob_pool", bufs=6) as ob_pool,
            tc.tile_pool(name="const_pool", bufs=1) as const_pool,
            tc.tile_pool(name="psAB", bufs=3, space="PSUM") as psAB_pool,
            tc.tile_pool(name="psO", bufs=2, space="PSUM") as psO_pool,
        ):
            # ---- PE warmup: dep-free small matmuls flip the HAM clock gate
            # (1.2->2.4GHz) while the first input DMAs are in flight, sized to
            # drain right as the first real matmul's data lands. --------------
            warm = const_pool.tile([P, P], BF16, tag="warm")
            nc.vector.memset(warm[:], 0.0)
            pswarm = psO_pool.tile([P, 512], F32, tag="psO", name="pswarm")
            for i in range(30):
                nc.tensor.matmul(pswarm[:, :P], warm[:], warm[:],
                                 start=(i == 0), stop=(i == 29))
            warmsink = const_pool.tile([P, 1], F32, tag="warmsink")
            nc.vector.tensor_scalar_mul(warmsink[:], pswarm[:, :1], 0.0)

            # ---- input DMAs, all on the SP HWDGE ring in the order compute
            # consumes them (just-in-time arrival; outputs use the ACT ring).
            w1k0t = const_pool.tile([P, P], BF16, tag="w1k0")
            nc.sync.dma_start(w1k0t[:], w1k0[:])

            w13ts = {}
            w13ts[0] = w13_pool.tile([P, 2 * KT * P], BF16, tag="w13",
                                     name="w13_0")
            nc.sync.dma_start(w13ts[0][:], w13p[0])

            # head x: k=0..3 as individual tiles (stagger arrivals with the
            # k-loop), k=4..7 as one DMA; then the tail groups in order.
            # Fewer issue slots (~592ns each on the serial ring) gets xg1's
            # transfer started ~2 slots earlier -- it's the critical arrival.
            xhs = []
            xgs = {}
            for k in range(4):
                xh = xh_pool.tile([P, HEAD], BF16, tag="xh", name=f"xh_{k}")
                nc.sync.dma_start(xh[:], xt2[:, k, :HEAD])
                xhs.append(xh)
            xh47 = xh_pool.tile([P, 4 * HEAD], BF16, tag="xh47")
            nc.sync.dma_start(xh47[:], xt2[:, 4:, :HEAD])
            def emit_w13(m):
                w13ts[m] = w13_pool.tile([P, 2 * KT * P], BF16, tag="w13",
                                         name=f"w13_{m}")
                nc.sync.dma_start(w13ts[m][:], w13p[m])

            for gi, (g0, gw) in enumerate(groups):
                if gi == 0:
                    continue
                xg = xg_pool.tile([P, KT * gw], BF16, tag=f"xg{gi}",
                                  name=f"xg_{gi}")
                nc.sync.dma_start(xg[:], xt2[:, :, g0:g0 + gw])
                xgs[gi] = xg
                if gi == 1:
                    # m=1's weights are consumed before xg2 under the
                    # interleaved (m, group) order below
                    emit_w13(1)

            cft = const_pool.tile([P, TT], F32, tag="cft")
            nc.sync.dma_start(cft[:], cf[:])

            for m in range(2, MT):
                emit_w13(m)

            w2t = w2_pool.tile([P, MT_BF * D], BF16, tag="w2a")
            nc.sync.dma_start(w2t[:], w2a[:])
            w2drt = w2_pool.tile([P, 2, D], F8, tag="w2dr")
            nc.sync.dma_start(w2drt[:], w2dr[:])

            def xt_slice(k, gi, gw):
                if gi == 0:
                    if k < 4:
                        return xhs[k][:, :gw]
                    return xh47[:, (k - 4) * HEAD:(k - 4) * HEAD + gw]
                return xgs[gi][:, k * gw:k * gw + gw]

            # --- phase 1: gT[h, tok] = relu(W1 @ xT)^2 * (W3 @ xT) ----------
            gts = []
            for m in range(MT_BF):
                gt = g_pool.tile([P, C], BF16, tag=f"g{m}", name=f"g_{m}")
                gts.append(gt)
            # h-tiles 14,15 as an fp8 pair (h scaled by 1/4; w2dr carries x4):
            # phase 2 contracts both in a single DoubleRow matmul
            gp = g_pool.tile([P, 2, CP], F8, tag="gpair")

            # unit order: m0/m1 run groups 0,1 first so the early PE work
            # only consumes data the DMA stream can deliver by then; groups
            # 2+ (the late xg arrivals) are consumed ~7us later than in the
            # plain m-major order, clearing the cumulative-bandwidth wall
            NG = len(groups)
            units = [(0, 0), (0, 1), (1, 0), (1, 1)]
            units += [(0, g) for g in range(2, NG)]
            units += [(1, g) for g in range(2, NG)]
            units += [(m, g) for m in range(2, MT) for g in range(NG)]
            for m, gi in units:
                w13t = w13ts[m]
                w1t = w13t[:, :KT * P]
                w3t = w13t[:, KT * P:]
                for gi, (g0, gw) in [(gi, groups[gi])]:
                    psA = psAB_pool.tile([P, 512], F32, tag="psA",
                                         name=f"psA_{m}_{g0}")
                    psB = psAB_pool.tile([P, 512], F32, tag="psB",
                                         name=f"psB_{m}_{g0}")
                    for k in range(KT):
                        w1s = (w1k0t[:] if m == 0 and k == 0
                               else w1t[:, k * P:(k + 1) * P])
                        nc.tensor.matmul(
                            psA[:, :gw],
                            w1s,
                            xt_slice(k, gi, gw),
                            start=(k == 0),
                            stop=(k == KT - 1),
                        )
                    for k in range(KT):
                        nc.tensor.matmul(
                            psB[:, :gw],
                            w3t[:, k * P:(k + 1) * P],
                            xt_slice(k, gi, gw),
                            start=(k == 0),
                            stop=(k == KT - 1),
                        )
                    r = tmp_pool.tile([P, 512], F32, tag="r",
                                      name=f"r_{m}_{g0}")
                    nc.vector.tensor_relu(r[:, :gw], psA[:, :gw])
                    t2 = tmp_pool.tile([P, 512], F32, tag="t2",
                                       name=f"t2_{m}_{g0}")
                    nc.vector.tensor_mul(t2[:, :gw], r[:, :gw], r[:, :gw])
                    if m < MT_BF:
                        nc.vector.tensor_mul(
                            gts[m][:, g0:g0 + gw],
                            t2[:, :gw],
                            psB[:, :gw],
                        )
                    else:
                        nc.vector.tensor_scalar_mul(t2[:, :gw], t2[:, :gw],
                                                    0.25)
                        nc.vector.tensor_mul(
                            gp[:, m - MT_BF, g0:g0 + gw],
                            t2[:, :gw],
                            psB[:, :gw],
                        )

            # --- phase 2: out[tok, d] = coef * (g.T @ W2T) ------------------
            def w2_slice(hk, dg0, dgw):
                return w2t[:, hk * D + dg0:hk * D + dg0 + dgw]

            for t in range(TT):
                tw = P if t < TT - 1 else TL
                for dg in range(2):
                    if t == TT - 1 and dg == 1:
                        # split the final accumulation into quarters so each
                        # piece's scale+store overlaps the PE's remaining
                        # matmuls; quarters land in the contiguous out2
                        for h in range(4):
                            pso = psO_pool.tile([P, 512], F32, tag="psO",
                                                name=f"psO_{t}_{dg}_{h}")
                            for hk in range(MT_BF):
                                nc.tensor.matmul(
                                    pso[:tw, :128],
                                    gts[hk][:, t * P:t * P + tw],
                                    w2_slice(hk, dg * 512 + h * 128, 128),
                                    start=(hk == 0),
                                    stop=False,
                                )
                            d0 = dg * 512 + h * 128
                            nc.tensor.matmul(
                                pso[:tw, :128],
                                gp[:, :, t * P:t * P + tw],
                                w2drt[:, :, d0:d0 + 128],
                                start=False,
                                stop=True,
                                perf_mode=mybir.MatmulPerfMode.DoubleRow,
                            )
                            ob = ob_pool.tile([P, 512], BF16, tag="ob",
                                              name=f"ob_{t}_{dg}_{h}")
                            nc.vector.tensor_scalar_mul(ob[:tw, :128],
                                                        pso[:tw, :128],
                                                        cft[:tw, t:t + 1])
                            # alternate rings so the last two issues (~840ns
                            # each) run in parallel instead of serializing
                            eng = nc.scalar if h % 2 == 0 else nc.sync
                            eng.dma_start(out2[h, :tw], ob[:tw, :128])
                        continue
                    pso = psO_pool.tile([P, 512], F32, tag="psO",
                                        name=f"psO_{t}_{dg}")
                    for hk in range(MT_BF):
                        nc.tensor.matmul(
                            pso[:tw, :],
                            gts[hk][:, t * P:t * P + tw],
                            w2_slice(hk, dg * 512, 512),
                            start=(hk == 0),
                            stop=False,
                        )
                    nc.tensor.matmul(
                        pso[:tw, :],
                        gp[:, :, t * P:t * P + tw],
                        w2drt[:, :, dg * 512:(dg + 1) * 512],
                        start=False,
                        stop=True,
                        perf_mode=mybir.MatmulPerfMode.DoubleRow,
                    )
                    ob = ob_pool.tile([P, 512], BF16, tag="ob",
                                      name=f"ob_{t}_{dg}")
                    nc.vector.tensor_scalar_mul(ob[:tw, :], pso[:tw, :],
                                                cft[:tw, t:t + 1])
                    nc.scalar.dma_start(out[t, dg, :tw], ob[:tw, :])

    if not nc.is_finalized():
        nc.finalize()
    return nc


def kernel(x, W1, W2, W3, gate_w, gate_b):
    global LAST_RESULTS

    xf = np.ascontiguousarray(x.reshape(N, D).astype(np.float32, copy=False))

    # ---- gate: softmax + top-2 (tiny, done on host) ------------------------
    logits = xf @ gate_w.T.astype(np.float32) + gate_b.astype(np.float32)
    logits -= logits.max(axis=-1, keepdims=True)
    probs = np.exp(logits)
    probs /= probs.sum(axis=-1, keepdims=True)
    order = np.argsort(-probs, axis=-1, kind="stable")
    i1, i2 = order[:, 0], order[:, 1]
    ar = np.arange(N)
    p1, p2 = probs[ar, i1], probs[ar, i2]
    ps = p1 + p2
    c1, c2 = p1 / ps, p2 / ps

    idx_list, coef_list = [], []
    for e in range(E):
        m1 = i1 == e
        m2 = i2 == e
        ide = np.nonzero(m1 | m2)[0]
        ce = np.where(m1[ide], c1[ide], c2[ide]).astype(np.float32)
        idx_list.append(ide)
        coef_list.append(ce)

    nmax = max(len(i) for i in idx_list)
    C = max(((nmax + 7) // 8) * 8, 512)
    TT = (C + P - 1) // P

    # ---- per-core input packing -------------------------------------------
    in_maps = []
    for e in range(E):
        ide, ce = idx_list[e], coef_list[e]
        ne = len(ide)

        xg = np.zeros((C, D), np.float32)
        xg[:ne] = xf[ide]
        # xt2[p, k, c] = x[c, k*128+p]
        xt2_np = np.ascontiguousarray(
            xg.T.reshape(KT, P, C).transpose(1, 0, 2)
        ).astype(BF16_NP)

        w1e = np.asarray(W1[e], np.float32)  # [H, D]
        w3e = np.asarray(W3[e], np.float32)  # [H, D]
        w2e = np.asarray(W2[e], np.float32)  # [D, H]
        # [m, h, k, d] -> [m, d, k, h] : packed[m][d, k*128+h] = W1[m*128+h, k*128+d]
        w1p_np = w1e.reshape(MT, P, KT, P).transpose(0, 3, 2, 1)
        w3p_np = w3e.reshape(MT, P, KT, P).transpose(0, 3, 2, 1)
        w13p_np = np.ascontiguousarray(
            np.stack([w1p_np, w3p_np], axis=2)   # [MT, P, 2, KT, P]
        ).reshape(MT, P, 2, KT * P).astype(BF16_NP)
        # w2a[p, hk*D + d] = W2[d, hk*128+p] for hk < 14
        w2a_np = np.ascontiguousarray(
            w2e.T.reshape(MT, P, D)[:MT_BF].transpose(1, 0, 2)
        ).reshape(P, MT_BF * D).astype(BF16_NP)
        # w2dr[p, j, d] = W2[d, (14+j)*128+p] * 4  (fp8)
        w2dr_np = np.ascontiguousarray(
            (w2e[:, MT_BF * P:] * 4.0).reshape(D, 2, P).transpose(2, 1, 0)
        ).astype(F8_NP)

        cfe = np.zeros(TT * P, np.float32)
        cfe[:ne] = ce
        cf_np = np.ascontiguousarray(cfe.reshape(TT, P).T)

        in_maps.append(
            {"xt2": xt2_np, "w13p": w13p_np, "w2a": w2a_np,
             "w2dr": w2dr_np, "cf": cf_np,
             "w1k0": np.ascontiguousarray(w13p_np[0, :, 0, :P])}
        )

    # ---- build + run on 8 cores -------------------------------------------
    nc = build_kernel(C)
    res = None
    last_exc = None
    for attempt in range(3):
        try:
            res = run_bass_kernel_spmd(
                nc, in_maps, core_ids=list(range(E)),
                trace=TRACE and attempt == 0,
            )
            break
        except Exception as exc:  # transient device wedge / trace plumbing
            last_exc = exc
    if res is None:
        raise last_exc
    LAST_RESULTS = res

    # ---- combine ----------------------------------------------------------
    out = np.zeros((N, D), np.float32)
    for e in range(E):
        ide = idx_list[e]
        oe = res.results[e]["out"].astype(np.float32)  # [TT, 2, P, 512]
        o2 = res.results[e]["out2"].astype(np.float32)  # [4, P, 128]
        # stitch the staged quarters back into the final (t, dg=1) tile
        oe[TT - 1, 1] = o2.transpose(1, 0, 2).reshape(P, 512)
        oe = oe.transpose(0, 2, 1, 3).reshape(TT * P, D)
        out[ide] += oe[: len(ide)]

    return out.reshape(B, S, D)


# revision 21
# speedup vs baseline: 1.0102x; 1.0102x over previous
"""MoE top-2 routed FFN (B=4, S=2048, D=1024, H=2048, E=8) on 8 TRN2 NeuronCores.

Strategy (expert-parallel, matching the sharding hint):
  - Host computes the tiny gate (softmax top-2) and builds per-expert token
    lists ("all-to-all dispatch" done at the sharding step).
  - Core e receives the tokens routed to expert e (gathered, transposed,
    zero-padded to capacity C), plus expert e's weights pre-packed into the
    exact tile layouts the kernel consumes.
  - Each core runs a dense FFN  out = coef * ((relu(x@W1.T)^2 * (x@W3.T)) @ W2.T)
    over its C tokens.  Matmuls run in bf16 with fp32 PSUM accumulation,
    except h-tiles 14,15 of the phase-2 contraction, which go through one
    fp8e4 DoubleRow matmul per output tile (2 contraction tiles per pass,
    ~2x rate; h scaled by 1/4 and W2 by 4 so the pair's partial product is
    exact in the shared accumulation).  Error scales with sqrt(fp8 fraction):
    one pair of 16 = measured end-to-end rel err 1.40e-2 (gate 2e-2);
    coefficients stay fp32, outputs bf16.
  - Host scatter-adds the per-expert outputs back ("combine").

Per-core kernel structure (single pass, weights read once):
  phase 1: for each of 16 H-tiles m: psA = W1m @ xT, psB = W3m @ xT (PSUM),
           gT[m] = relu(psA)^2 * psB  (DVE, bf16)   [H, C] layout
  phase 2: for each 128-token tile: out[tok, :] = (gT.T @ W2T) * coef  (PSUM->DVE->DRAM)

Perf notes:
  - Dep-free warmup (36 N=128 matmuls off a memset tile) starts at ~7us and
    flips the HAM clock gate (1.2->2.4GHz) while the first input DMAs are in
    flight; sized to drain right as the first real matmul's data lands.
  - Few, large input DMAs on the SP HWDGE ring in consumption order (W1|W3
    packed per m-tile, x tail groups as single 3D DMAs, W2 as one 4MB DMA).
    Concurrent DMA queues share SDMA bandwidth equally, so later-needed
    groups are interleaved to not starve the group-0 head path; w13 pool
    reuse (bufs=3) compute-paces the later weight DMAs on the serial ring.
  - Output DMAs ride the otherwise-idle ACT HWDGE ring, in bf16, and the
    final token tile's quarter-pieces land in a contiguous staging tensor
    (strided [:,h*128:(h+1)*128] DRAM writes were descriptor-bound, ~9us of
    tail); ob_pool bufs=6 avoids a WAR stall on the last quarter's scale.
"""

import os
import sys

import numpy as np

if os.path.isdir("/opt/trn_rl_repo") and "/opt/trn_rl_repo" not in sys.path:
    sys.path.insert(0, "/opt/trn_rl_repo")

import ml_dtypes

import concourse.bacc as bacc
import concourse.mybir as mybir
from concourse.bass_utils import run_bass_kernel_spmd
from concourse.tile import TileContext

B, S, D, H, E = 4, 2048, 1024, 2048, 8
N = B * S
P = 128
KT = D // P   # 8 contraction tiles over D
MT = H // P   # 16 tiles over H

F32 = mybir.dt.float32
BF16 = mybir.dt.bfloat16
F8 = mybir.dt.float8e4
BF16_NP = ml_dtypes.bfloat16
F8_NP = ml_dtypes.float8_e4m3fn
MT_BF = 14   # h-tiles 0..13 stay bf16; tiles 14,15 go fp8 via one DoubleRow MM

# Set by test harness to capture profiling info.
TRACE = False
LAST_RESULTS = None


def _token_groups(c0, cw):
    """Split [c0, c0+cw) into moving-dim groups of at most 512."""
    groups = []
    rem = cw
    off = c0
    while rem > 0:
        if 512 < rem < 768:
            g = max(min(rem - 256, 512), 256)
        else:
            g = min(512, rem)
        groups.append((off, g))
        off += g
        rem -= g
    return groups


def build_kernel(C):
    TT = (C + P - 1) // P
    TL = C - (TT - 1) * P  # last token tile width (<=128)
    CP = ((C + 15) // 16) * 16  # fp8 pair-tile stride (step%16==0)
    groups = _token_groups(0, C)
    HEAD = groups[0][1]  # first group, served by per-k head tiles

    nc = bacc.Bacc("TRN2", target_bir_lowering=False)

    # x transposed, token-major-last: xt2[p, k, c] = x_tok[c, k*128+p]
    xt2 = nc.dram_tensor("xt2", [P, KT, C], BF16, kind="ExternalInput")
    # w13p[m][d, 0, k*128+h] = W1[m*128+h, k*128+d]; [:,1,:] same for W3
    w13p = nc.dram_tensor("w13p", [MT, P, 2, KT * P], BF16, kind="ExternalInput")
    # 32KB duplicate of w13p[0,:,0,:128] so the first matmul isn't gated on
    # the full 512KB m=0 weight transfer
    w1k0 = nc.dram_tensor("w1k0", [P, P], BF16, kind="ExternalInput")
    # w2a[p, hk, d] = W2[d, hk*128+p]  (bf16 h-tiles 0..13 only)
    w2a = nc.dram_tensor("w2a", [P, MT_BF * D], BF16, kind="ExternalInput")
    # fp8 pair for h-tiles 14,15: w2dr[p, j, d] = W2[d, (14+j)*128+p] * 4
    w2dr = nc.dram_tensor("w2dr", [P, 2, D], F8, kind="ExternalInput")
    cf = nc.dram_tensor("cf", [P, TT], F32, kind="ExternalInput")
    out = nc.dram_tensor("out", [TT, 2, P, 512], BF16, kind="ExternalOutput")
    # contiguous staging for the final (t, dg=1) quarter pieces
    out2 = nc.dram_tensor("out2", [4, P, 128], BF16, kind="ExternalOutput")

    with TileContext(nc) as tc:
        with (
            tc.tile_pool(name="xh_pool", bufs=KT) as xh_pool,
            tc.tile_pool(name="xg_pool", bufs=1) as xg_pool,
            tc.tile_pool(name="g_pool", bufs=1) as g_pool,
            tc.tile_pool(name="w13_pool", bufs=3) as w13_pool,
            tc.tile_pool(name="w2_pool", bufs=1) as w2_pool,
            tc.tile_pool(name="tmp_pool", bufs=2) as tmp_pool,
            tc.tile_pool(name="ob_pool", bufs=6) as ob_pool,
            tc.tile_pool(name="const_pool", bufs=1) as const_pool,
            tc.tile_pool(name="psAB", bufs=3, space="PSUM") as psAB_pool,
            tc.tile_pool(name="psO", bufs=2, space="PSUM") as psO_pool,
        ):
            # ---- PE warmup: dep-free small matmuls flip the HAM clock gate
            # (1.2->2.4GHz) while the first input DMAs are in flight, sized to
            # drain right as the first real matmul's data lands. --------------
            warm = const_pool.tile([P, P], BF16, tag="warm")
            nc.vector.memset(warm[:], 0.0)
            pswarm = psO_pool.tile([P, 512], F32, tag="psO", name="pswarm")
            for i in range(30):
                nc.tensor.matmul(pswarm[:, :P], warm[:], warm[:],
                                 start=(i == 0), stop=(i == 29))
            warmsink = const_pool.tile([P, 1], F32, tag="warmsink")
            nc.vector.tensor_scalar_mul(warmsink[:], pswarm[:, :1], 0.0)

            # ---- input DMAs, all on the SP HWDGE ring in the order compute
            # consumes them (just-in-time arrival; outputs use the ACT ring).
            w1k0t = const_pool.tile([P, P], BF16, tag="w1k0")
            nc.sync.dma_start(w1k0t[:], w1k0[:])

            w13ts = {}
            w13ts[0] = w13_pool.tile([P, 2 * KT * P], BF16, tag="w13",
                                     name="w13_0")
            nc.sync.dma_start(w13ts[0][:], w13p[0])

            # head x: k=0..3 as individual tiles (stagger arrivals with the
            # k-loop), k=4..7 as one DMA; then the tail groups in order.
            # Fewer issue slots (~592ns each on the serial ring) gets xg1's
            # transfer started ~2 slots earlier -- it's the critical arrival.
            xhs = []
            xgs = {}
            for k in range(4):
                xh = xh_pool.tile([P, HEAD], BF16, tag="xh", name=f"xh_{k}")
                nc.sync.dma_start(xh[:], xt2[:, k, :HEAD])
                xhs.append(xh)
            xh47 = xh_pool.tile([P, 4 * HEAD], BF16, tag="xh47")
            nc.sync.dma_start(xh47[:], xt2[:, 4:, :HEAD])
            for gi, (g0, gw) in enumerate(groups):
                if gi == 0:
                    continue
                xg = xg_pool.tile([P, KT * gw], BF16, tag=f"xg{gi}",
                                  name=f"xg_{gi}")
                nc.sync.dma_start(xg[:], xt2[:, :, g0:g0 + gw])
                xgs[gi] = xg

            cft = const_pool.tile([P, TT], F32, tag="cft")
            nc.sync.dma_start(cft[:], cf[:])

            for m in range(1, MT):
                w13ts[m] = w13_pool.tile([P, 2 * KT * P], BF16, tag="w13",
                                         name=f"w13_{m}")
                nc.sync.dma_start(w13ts[m][:], w13p[m])

            w2t = w2_pool.tile([P, MT_BF * D], BF16, tag="w2a")
            nc.sync.dma_start(w2t[:], w2a[:])
            w2drt = w2_pool.tile([P, 2, D], F8, tag="w2dr")
            nc.sync.dma_start(w2drt[:], w2dr[:])

            def xt_slice(k, gi, gw):
                if gi == 0:
                    if k < 4:
                        return xhs[k][:, :gw]
                    return xh47[:, (k - 4) * HEAD:(k - 4) * HEAD + gw]
                return xgs[gi][:, k * gw:k * gw + gw]

            # --- phase 1: gT[h, tok] = relu(W1 @ xT)^2 * (W3 @ xT) ----------
            gts = []
            for m in range(MT_BF):
                gt = g_pool.tile([P, C], BF16, tag=f"g{m}", name=f"g_{m}")
                gts.append(gt)
            # h-tiles 14,15 as an fp8 pair (h scaled by 1/4; w2dr carries x4):
            # phase 2 contracts both in a single DoubleRow matmul
            gp = g_pool.tile([P, 2, CP], F8, tag="gpair")

            for m in range(MT):
                w13t = w13ts[m]
                w1t = w13t[:, :KT * P]
                w3t = w13t[:, KT * P:]
                for gi, (g0, gw) in enumerate(groups):
                    psA = psAB_pool.tile([P, 512], F32, tag="psA",
                                         name=f"psA_{m}_{g0}")
                    psB = psAB_pool.tile([P, 512], F32, tag="psB",
                                         name=f"psB_{m}_{g0}")
                    for k in range(KT):
                        w1s = (w1k0t[:] if m == 0 and k == 0
                               else w1t[:, k * P:(k + 1) * P])
                        nc.tensor.matmul(
                            psA[:, :gw],
                            w1s,
                            xt_slice(k, gi, gw),
                            start=(k == 0),
                            stop=(k == KT - 1),
                        )
                    for k in range(KT):
                        nc.tensor.matmul(
                            psB[:, :gw],
                            w3t[:, k * P:(k + 1) * P],
                            xt_slice(k, gi, gw),
                            start=(k == 0),
                            stop=(k == KT - 1),
                        )
                    r = tmp_pool.tile([P, 512], F32, tag="r",
                                      name=f"r_{m}_{g0}")
                    nc.vector.tensor_relu(r[:, :gw], psA[:, :gw])
                    t2 = tmp_pool.tile([P, 512], F32, tag="t2",
                                       name=f"t2_{m}_{g0}")
                    nc.vector.tensor_mul(t2[:, :gw], r[:, :gw], r[:, :gw])
                    if m < MT_BF:
                        nc.vector.tensor_mul(
                            gts[m][:, g0:g0 + gw],
                            t2[:, :gw],
                            psB[:, :gw],
                        )
                    else:
                        nc.vector.tensor_scalar_mul(t2[:, :gw], t2[:, :gw],
                                                    0.25)
                        nc.vector.tensor_mul(
                            gp[:, m - MT_BF, g0:g0 + gw],
                            t2[:, :gw],
                            psB[:, :gw],
                        )

            # --- phase 2: out[tok, d] = coef * (g.T @ W2T) ------------------
            def w2_slice(hk, dg0, dgw):
                return w2t[:, hk * D + dg0:hk * D + dg0 + dgw]

            for t in range(TT):
                tw = P if t < TT - 1 else TL
                for dg in range(2):
                    if t == TT - 1 and dg == 1:
                        # split the final accumulation into quarters so each
                        # piece's scale+store overlaps the PE's remaining
                        # matmuls; quarters land in the contiguous out2
                        for h in range(4):
                            pso = psO_pool.tile([P, 512], F32, tag="psO",
                                                name=f"psO_{t}_{dg}_{h}")
                            for hk in range(MT_BF):
                                nc.tensor.matmul(
                                    pso[:tw, :128],
                                    gts[hk][:, t * P:t * P + tw],
                                    w2_slice(hk, dg * 512 + h * 128, 128),
                                    start=(hk == 0),
                                    stop=False,
                                )
                            d0 = dg * 512 + h * 128
                            nc.tensor.matmul(
                                pso[:tw, :128],
                                gp[:, :, t * P:t * P + tw],
                                w2drt[:, :, d0:d0 + 128],
                                start=False,
                                stop=True,
                                perf_mode=mybir.MatmulPerfMode.DoubleRow,
                            )
                            ob = ob_pool.tile([P, 512], BF16, tag="ob",
                                              name=f"ob_{t}_{dg}_{h}")
                            nc.vector.tensor_scalar_mul(ob[:tw, :128],
                                                        pso[:tw, :128],
                                                        cft[:tw, t:t + 1])
                            # alternate rings so the last two issues (~840ns
                            # each) run in parallel instead of serializing
                            eng = nc.scalar if h % 2 == 0 else nc.sync
                            eng.dma_start(out2[h, :tw], ob[:tw, :128])
                        continue
                    pso = psO_pool.tile([P, 512], F32, tag="psO",
                                        name=f"psO_{t}_{dg}")
                    for hk in range(MT_BF):
                        nc.tensor.matmul(
                            pso[:tw, :],
                            gts[hk][:, t * P:t * P + tw],
                            w2_slice(hk, dg * 512, 512),
                            start=(hk == 0),
                            stop=False,
                        )
                    nc.tensor.matmul(
                        pso[:tw, :],
                        gp[:, :, t * P:t * P + tw],
                        w2drt[:, :, dg * 512:(dg + 1) * 512],
                        start=False,
                        stop=True,
                        perf_mode=mybir.MatmulPerfMode.DoubleRow,
                    )
                    ob = ob_pool.tile([P, 512], BF16, tag="ob",
                                      name=f"ob_{t}_{dg}")
                    nc.vector.tensor_scalar_mul(ob[:tw, :], pso[:tw, :],
                                                cft[:tw, t:t + 1])
                    nc.scalar.dma_start(out[t, dg, :tw], ob[:tw, :])

    if not nc.is_finalized():
        nc.finalize()
    return nc


def kernel(x, W1, W2, W3, gate_w, gate_b):
    global LAST_RESULTS

    xf = np.ascontiguousarray(x.reshape(N, D).astype(np.float32, copy=False))

    # ---- gate: softmax + top-2 (tiny, done on host) ------------------------
    logits = xf @ gate_w.T.astype(np.float32) + gate_b.astype(np.float32)
    logits -= logits.max(axis=-1, keepdims=True)
    probs = np.exp(logits)
    probs /= probs.sum(axis=-1, keepdims=True)
    order = np.argsort(-probs, axis=-1, kind="stable")
    i1, i2 = order[:, 0], order[:, 1]
    ar = np.arange(N)
    p1, p2 = probs[ar, i1], probs[ar, i2]
    ps = p1 + p2
    c1, c2 = p1 / ps, p2 / ps

    idx_list, coef_list = [], []
    for e in range(E):
        m1 = i1 == e
        m2 = i2 == e
        ide = np.nonzero(m1 | m2)[0]
        ce = np.where(m1[ide], c1[ide], c2[ide]).astype(np.float32)
        idx_list.append(ide)
        coef_list.append(ce)

    nmax = max(len(i) for i in idx_list)
    C = max(((nmax + 7) // 8) * 8, 512)
    TT = (C + P - 1) // P

    # ---- per-core input packing -------------------------------------------
    in_maps = []
    for e in range(E):
        ide, ce = idx_list[e], coef_list[e]
        ne = len(ide)

        xg = np.zeros((C, D), np.float32)
        xg[:ne] = xf[ide]
        # xt2[p, k, c] = x[c, k*128+p]
        xt2_np = np.ascontiguousarray(
            xg.T.reshape(KT, P, C).transpose(1, 0, 2)
        ).astype(BF16_NP)

        w1e = np.asarray(W1[e], np.float32)  # [H, D]
        w3e = np.asarray(W3[e], np.float32)  # [H, D]
        w2e = np.asarray(W2[e], np.float32)  # [D, H]
        # [m, h, k, d] -> [m, d, k, h] : packed[m][d, k*128+h] = W1[m*128+h, k*128+d]
        w1p_np = w1e.reshape(MT, P, KT, P).transpose(0, 3, 2, 1)
        w3p_np = w3e.reshape(MT, P, KT, P).transpose(0, 3, 2, 1)
        w13p_np = np.ascontiguousarray(
            np.stack([w1p_np, w3p_np], axis=2)   # [MT, P, 2, KT, P]
        ).reshape(MT, P, 2, KT * P).astype(BF16_NP)
        # w2a[p, hk*D + d] = W2[d, hk*128+p] for hk < 14
        w2a_np = np.ascontiguousarray(
            w2e.T.reshape(MT, P, D)[:MT_BF].transpose(1, 0, 2)
        ).reshape(P, MT_BF * D).astype(BF16_NP)
        # w2dr[p, j, d] = W2[d, (14+j)*128+p] * 4  (fp8)
        w2dr_np = np.ascontiguousarray(
            (w2e[:, MT_BF * P:] * 4.0).reshape(D, 2, P).transpose(2, 1, 0)
        ).astype(F8_NP)

        cfe = np.zeros(TT * P, np.float32)
        cfe[:ne] = ce
        cf_np = np.ascontiguousarray(cfe.reshape(TT, P).T)

        in_maps.append(
            {"xt2": xt2_np, "w13p": w13p_np, "w2a": w2a_np,
             "w2dr": w2dr_np, "cf": cf_np,
             "w1k0": np.ascontiguousarray(w13p_np[0, :, 0, :P])}
        )

    # ---- build + run on 8 cores -------------------------------------------
    nc = build_kernel(C)
    res = None
    last_exc = None
    for attempt in range(3):
        try:
            res = run_bass_kernel_spmd(
                nc, in_maps, core_ids=list(range(E)),
                trace=TRACE and attempt == 0,
            )
            break
        except Exception as exc:  # transient device wedge / trace plumbing
            last_exc = exc
    if res is None:
        raise last_exc
    LAST_RESULTS = res

    # ---- combine ----------------------------------------------------------
    out = np.zeros((N, D), np.float32)
    for e in range(E):
        ide = idx_list[e]
        oe = res.results[e]["out"].astype(np.float32)  # [TT, 2, P, 512]
        o2 = res.results[e]["out2"].astype(np.float32)  # [4, P, 128]
        # stitch the staged quarters back into the final (t, dg=1) tile
        oe[TT - 1, 1] = o2.transpose(1, 0, 2).reshape(P, 512)
        oe = oe.transpose(0, 2, 1, 3).reshape(TT * P, D)
        out[ide] += oe[: len(ide)]

    return out.reshape(B, S, D)
